# revision 7
# baseline (speedup 1.0000x reference)
"""Bass/Tile TRN2 kernel for nn_CrossAttentionLayer (B=8, NQ=64, S=4096, D=1024, H=16).

Sharding: pure data-parallel — core b computes batch element b. No collectives.

Math per core (x = queries[b] (64,1024), C = context[b] (4096,1024)):
    Q = x @ wq.T + bq                         (setup)
    A = Q @ wk          per head              (setup; bk cancels in softmax — exact)
    scoresT[s, (h,q)] = C @ A.T               (big GEMM 1)
    p = exp(scoresT * SCALE)                  (no max-subtraction; logits are O(8))
    UT[d, (h,q)] = C.T @ p                    (big GEMM 2 — attention over RAW context)
    r[(h,q)] = sum_s p                        (DVE reduce + gpsimd partition_all_reduce)
    OT_pair = wv_pair @ UT_pair               (small GEMM), then *1/r, +bv
    out = O @ wo.T + bo

Why this shape: everything is bf16 (fp8 fails the 2e-2 gate: softmax-weight noise
propagates ~1:1 to the output), and the PE cost on TRN2 is output-free-dim rows
only. Absorbing wk into A kills the QK^T matmuls; attending over raw C and
projecting through wv AFTERWARDS kills the separate PV pass and the transposes.
PE rows/body: scores 109us + PC 109us + wv-proj 3.4us + out-proj 3.4us ~= 219us
vs baseline ~251us. Rowsums/normalization ride the otherwise-idle DVE/GpSimd.
"""

import numpy as np
import ml_dtypes
from contextlib import ExitStack

import concourse.bass as bass
import concourse.tile as tile
import concourse.bass_isa as bass_isa
from concourse import bacc, mybir
from concourse.bass_utils import run_bass_kernel_spmd

# problem constants (hardcoded per contract)
B, NQ, S, D = 8, 64, 4096, 1024
H, HD = 16, 64
N_CORES = 8
SCALE = float(HD) ** -0.5

BF16 = mybir.dt.bfloat16
F32 = mybir.dt.float32
NPBF16 = ml_dtypes.bfloat16
AF = mybir.ActivationFunctionType

S_BLK = 1024
N_BLK = S // S_BLK
N_SUB = S_BLK // 128   # 128-row s-subtiles per block
DT = D // 128          # 128-wide d-tiles
NPAIR = H // 2         # head pairs (two 64-dim heads per 128-partition tile)

_PROGRAM = None


def _emit(ctx: ExitStack, tc: tile.TileContext, aps: dict, dbg: dict | None = None, repeat: int = 1):
    nc = tc.nc
    qT = aps["qT"]
    wqT, wkn, wvT, woT = aps["wqT"], aps["wkn"], aps["wvT"], aps["woT"]
    bqr, bvr, bor = aps["bqr"], aps["bvr"], aps["bor"]

    const = ctx.enter_context(tc.tile_pool(name="const", bufs=1))
    wpool = ctx.enter_context(tc.tile_pool(name="wpool", bufs=1))
    # ctxT pool also hosts the setup-only wq/wk tiles (same shape); they get
    # recycled by the first body's ctx loads once Q/A are done.
    ctxp = ctx.enter_context(tc.tile_pool(name="ctxp", bufs=16))
    ctxn = ctx.enter_context(tc.tile_pool(name="ctxn", bufs=10))
    expp = ctx.enter_context(tc.tile_pool(name="expp", bufs=3))
    work = ctx.enter_context(tc.tile_pool(name="work", bufs=1))
    psum_sc = ctx.enter_context(tc.tile_pool(name="psum_sc", bufs=3, space="PSUM"))
    psum_u = ctx.enter_context(tc.tile_pool(name="psum_u", bufs=3, space="PSUM"))
    psum_ms = ctx.enter_context(tc.tile_pool(name="psum_ms", bufs=2, space="PSUM"))

    # ---- weight loads ----
    def load_w(pool, name, dram, tag=None):
        tiles = [
            pool.tile([128, D], BF16, tag=tag or f"{name}{t}", name=f"{name}{t}")
            for t in range(DT)
        ]
        for t in range(DT):
            nc.sync.dma_start(tiles[t][:], dram[128 * t : 128 * (t + 1), :])
        return tiles

    wv_sb = load_w(wpool, "wv", wvT)
    wo_sb = load_w(wpool, "wo", woT)
    wq_sb = load_w(ctxp, "wq", wqT, tag="ctx")   # setup-only, recycled
    wk_sb = load_w(ctxp, "wk", wkn, tag="ctx")   # setup-only, recycled

    qt_sb = [const.tile([128, NQ], BF16, tag=f"qt{t}", name=f"qt{t}") for t in range(DT)]
    for t in range(DT):
        nc.sync.dma_start(qt_sb[t][:], qT[128 * t : 128 * (t + 1), :])

    bq_sb = const.tile([128, DT], F32, tag="bq", name="bq_sb")
    nc.sync.dma_start(bq_sb[:], bqr[:, :])
    bv_sb = const.tile([128, DT], F32, tag="bv", name="bv_sb")
    nc.sync.dma_start(bv_sb[:], bvr[:, :])
    bo_sb = const.tile([1, D], F32, tag="bo", name="bo_sb")
    nc.sync.dma_start(bo_sb[:], bor[:, :])

    ones_row = const.tile([1, 128], F32, tag="ones_row", name="ones_row")
    nc.vector.memset(ones_row[:], 1.0)

    # bo broadcast rows via ones-column matmul (one-time)
    bo_bc = const.tile([NQ, D], F32, tag="bo_bc", name="bo_bc")
    for c in range(2):
        ps = psum_ms.tile([NQ, 512], F32, tag="ms", name="ms_ps")
        nc.tensor.matmul(
            ps[:], ones_row[:, 0:NQ], bo_sb[:, 512 * c : 512 * (c + 1)],
            start=True, stop=True,
        )
        nc.vector.tensor_copy(bo_bc[:, 512 * c : 512 * (c + 1)], ps[:])

    # ---- Q projection, packed block-diagonally per head pair ----
    # qt2[t] = [[Q_{2t}^T, 0], [0, Q_{2t+1}^T]]  (128 x 128, bf16)
    qt2 = []
    for t in range(DT):
        q2 = const.tile([128, 128], BF16, tag=f"qt2_{t}", name=f"qt2_{t}")
        nc.vector.memset(q2[:], 0.0)
        ps = psum_ms.tile([128, NQ], F32, tag="ms", name="ms_ps")
        for d in range(DT):
            nc.tensor.matmul(
                ps[:],
                wq_sb[d][:, 128 * t : 128 * (t + 1)],
                qt_sb[d][:],
                start=(d == 0),
                stop=(d == DT - 1),
            )
        nc.scalar.activation(
            q2[0:64, 0:64], ps[0:64, :], AF.Identity, bias=bq_sb[0:64, t : t + 1]
        )
        nc.scalar.activation(
            q2[64:128, 64:128], ps[64:128, :], AF.Identity, bias=bq_sb[64:128, t : t + 1]
        )
        qt2.append(q2)

    # ---- A = Q @ wk, stored transposed bf16: atb[dc][din-local, (h,q)] ----
    # qt2[t] is block-diagonal, so k-tile t feeds exactly (h,q) cols 128t..128t+128.
    atb = [wpool.tile([128, D], BF16, tag=f"atb{k}", name=f"atb{k}") for k in range(DT)]
    for dc in range(DT):
        for c in range(2):
            ps = psum_ms.tile([128, 512], F32, tag="ms", name="ms_ps")
            for j in range(4):
                t = 4 * c + j
                nc.tensor.matmul(
                    ps[:, 128 * j : 128 * (j + 1)],
                    wk_sb[t][:, 128 * dc : 128 * (dc + 1)],
                    qt2[t][:],
                    start=(j == 0),
                    stop=(j == 3),
                )
            nc.vector.tensor_copy(atb[dc][:, 512 * c : 512 * (c + 1)], ps[:])

    if dbg is not None:
        nc.sync.dma_start(dbg["qt2_0"][:, :], qt2[0][:])
        nc.sync.dma_start(dbg["atb0"][:, :], atb[0][:])

    # ---- persistent accumulators ----
    # UT[dc][c]: f32 [128, 512]; racc: replicated rowsums [128, 1024] f32
    ut = [[const.tile([128, 512], F32, tag=f"ut{dc}_{c}", name=f"ut{dc}_{c}") for c in range(2)] for dc in range(DT)]
    utb = [const.tile([128, D], BF16, tag=f"utb{dc}", name=f"utb{dc}") for dc in range(DT)]
    racc = const.tile([128, D], F32, tag="racc", name="racc")

    for _rep in range(repeat):
        _emit_body(tc, aps, dbg if _rep == 0 else None, locals())


def _emit_body(tc: tile.TileContext, aps: dict, dbg: dict | None, env: dict):
    nc = tc.nc
    ctxT, ctxN, out = aps["ctxT"], aps["ctxN"], aps["out"]
    ctxp, ctxn, expp, work = env["ctxp"], env["ctxn"], env["expp"], env["work"]
    psum_sc, psum_u, psum_ms = env["psum_sc"], env["psum_u"], env["psum_ms"]
    wv_sb, wo_sb, atb = env["wv_sb"], env["wo_sb"], env["atb"]
    bv_sb, bo_bc = env["bv_sb"], env["bo_bc"]
    ut, utb, racc = env["ut"], env["utb"], env["racc"]
    const, qt2 = env["const"], env["qt2"]

    for blk in range(N_BLK):
        s0 = blk * S_BLK
        first, last = blk == 0, blk == N_BLK - 1
        ctxT_sb = [ctxp.tile([128, S_BLK], BF16, tag="ctx", name="ctxT_t") for _ in range(DT)]
        for dc in range(DT):
            nc.sync.dma_start(
                ctxT_sb[dc][:], ctxT[128 * dc : 128 * (dc + 1), s0 : s0 + S_BLK]
            )
        ctxN_sb = [ctxn.tile([128, D], BF16, tag="cn", name="ctxN_t") for _ in range(N_SUB)]
        for si in range(N_SUB):
            r0 = s0 + 128 * si
            nc.sync.dma_start(ctxN_sb[si][:], ctxN[r0 : r0 + 128, :])

        # scoresT GEMM + exp(bf16). expg[c][p, si, n] = exp weight for
        # s = s0+128*si+p, (h,q) column 512c+n.
        expg = [expp.tile([128, N_SUB, 512], BF16, tag="exp", name="exp_t") for _ in range(2)]
        for si in range(N_SUB):
            for c in range(2):
                ps = psum_sc.tile([128, 512], F32, tag="sc", name="sc_ps")
                for dc in range(DT):
                    nc.tensor.matmul(
                        ps[:],
                        ctxT_sb[dc][:, 128 * si : 128 * (si + 1)],
                        atb[dc][:, 512 * c : 512 * (c + 1)],
                        start=(dc == 0),
                        stop=(dc == DT - 1),
                    )
                nc.scalar.activation(expg[c][:, si, :], ps[:], AF.Exp, scale=SCALE)

        # rowsum partials: DVE reduce over si, gpsimd all-reduce over s-partitions
        for c in range(2):
            rpart = work.tile([128, 512], F32, tag="rp", name="rpart")
            nc.vector.tensor_reduce(
                rpart[:],
                expg[c][:, :, :].rearrange("p a b -> p b a"),
                mybir.AxisListType.X,
                mybir.AluOpType.add,
            )
            radd = work.tile([128, 512], F32, tag="ra", name="radd")
            nc.gpsimd.partition_all_reduce(
                radd[:], rpart[:], channels=128, reduce_op=bass_isa.ReduceOp.add
            )
            if first:
                nc.gpsimd.tensor_copy(racc[:, 512 * c : 512 * (c + 1)], radd[:])
            else:
                nc.gpsimd.tensor_add(
                    racc[:, 512 * c : 512 * (c + 1)], radd[:],
                    racc[:, 512 * c : 512 * (c + 1)],
                )

        # PC GEMM: UT[dc][c] (+)= ctxN^T @ exp
        for dc in range(DT):
            for c in range(2):
                ps = psum_u.tile([128, 512], F32, tag="u", name="u_ps")
                for si in range(N_SUB):
                    nc.tensor.matmul(
                        ps[:],
                        ctxN_sb[si][:, 128 * dc : 128 * (dc + 1)],
                        expg[c][:, si, :],
                        start=(si == 0),
                        stop=(si == N_SUB - 1),
                    )
                if first:
                    nc.vector.tensor_copy(ut[dc][c][:], ps[:])
                elif last:
                    # final add writes the bf16 copy the wv-proj consumes
                    nc.vector.tensor_add(
                        utb[dc][:, 512 * c : 512 * (c + 1)], ps[:], ut[dc][c][:]
                    )
                else:
                    nc.vector.tensor_add(ut[dc][c][:], ps[:], ut[dc][c][:])

        if dbg is not None and blk == 0:
            nc.sync.dma_start(dbg["exp0"][:, :], expg[0][:, 0, :])

    bv_sb = env["bv_sb"]

    # ---- rowsum reciprocal, broadcast ----
    recip = work.tile([1, D], F32, tag="rc", name="recip")
    nc.vector.reciprocal(recip[:], racc[0:1, :])
    recip_bc = const.tile([128, D], F32, tag="rcbc", name="recip_bc")
    nc.gpsimd.partition_broadcast(recip_bc[:], recip[:])

    if dbg is not None:
        nc.sync.dma_start(dbg["racc"][:, :], racc[0:1, :])
        nc.sync.dma_start(dbg["ut0"][:, :], utb[0][:])

    # ---- wv-projection per pair -> OT, normalize, +bv, pack out-proj lhsT ----
    ots = []
    for p in range(NPAIR):
        ps = psum_ms.tile([128, 128], F32, tag="ms", name="ms_ps")
        for dc in range(DT):
            nc.tensor.matmul(
                ps[:],
                wv_sb[dc][:, 128 * p : 128 * (p + 1)],
                utb[dc][:, 128 * p : 128 * (p + 1)],
                start=(dc == 0),
                stop=(dc == DT - 1),
            )
        ot = const.tile([128, NQ], BF16, tag=f"ot{p}", name=f"ot{p}")
        om = work.tile([128, NQ], F32, tag="om", name="om_t")
        for h in range(2):
            sl = slice(64 * h, 64 * (h + 1))
            nc.vector.tensor_mul(
                om[sl, :], ps[sl, sl], recip_bc[sl, 128 * p + 64 * h : 128 * p + 64 * (h + 1)]
            )
            nc.scalar.activation(
                ot[sl, :], om[sl, :], AF.Identity, bias=bv_sb[sl, p : p + 1]
            )
        ots.append(ot)
        if dbg is not None and p == 0:
            nc.sync.dma_start(dbg["ot0"][:, :], ot[:])

    # ---- output projection ----
    out_sb = const.tile([NQ, D], F32, tag="out_sb", name="out_sb")
    for c in range(2):
        ps = psum_ms.tile([NQ, 512], F32, tag="ms", name="ms_ps")
        for t in range(DT):
            nc.tensor.matmul(
                ps[:],
                ots[t][:],
                wo_sb[t][:, 512 * c : 512 * (c + 1)],
                start=(t == 0),
                stop=(t == DT - 1),
            )
        nc.vector.tensor_add(
            out_sb[:, 512 * c : 512 * (c + 1)], ps[:], bo_bc[:, 512 * c : 512 * (c + 1)]
        )
    nc.sync.dma_start(out[:, :], out_sb[:])


DBG_SHAPES = {
    "qt2_0": ([128, 128], BF16),
    "atb0": ([128, D], BF16),
    "exp0": ([128, 512], BF16),
    "racc": ([1, D], F32),
    "ut0": ([128, D], BF16),
    "ot0": ([128, NQ], BF16),
}


def _build_program(debug_dumps: bool = False, repeat: int = 1):
    nc = bacc.Bacc("TRN2", target_bir_lowering=False, debug=False)
    aps = {
        "ctxT": nc.dram_tensor("ctxT", [D, S], BF16, kind="ExternalInput").ap(),
        "ctxN": nc.dram_tensor("ctxN", [S, D], BF16, kind="ExternalInput").ap(),
        "qT": nc.dram_tensor("qT", [D, NQ], BF16, kind="ExternalInput").ap(),
        "wqT": nc.dram_tensor("wqT", [D, D], BF16, kind="ExternalInput").ap(),
        "wkn": nc.dram_tensor("wkn", [D, D], BF16, kind="ExternalInput").ap(),
        "wvT": nc.dram_tensor("wvT", [D, D], BF16, kind="ExternalInput").ap(),
        "woT": nc.dram_tensor("woT", [D, D], BF16, kind="ExternalInput").ap(),
        "bqr": nc.dram_tensor("bqr", [128, DT], F32, kind="ExternalInput").ap(),
        "bvr": nc.dram_tensor("bvr", [128, DT], F32, kind="ExternalInput").ap(),
        "bor": nc.dram_tensor("bor", [1, D], F32, kind="ExternalInput").ap(),
        "out": nc.dram_tensor("out", [NQ, D], F32, kind="ExternalOutput").ap(),
    }
    dbg = None
    if debug_dumps:
        dbg = {
            k: nc.dram_tensor(f"dbg_{k}", shp, dt, kind="ExternalOutput").ap()
            for k, (shp, dt) in DBG_SHAPES.items()
        }
    with tile.TileContext(nc) as tc:
        with ExitStack() as stack:
            _emit(stack, tc, aps, dbg, repeat=repeat)
    nc.compile()
    return nc


def _get_program():
    global _PROGRAM
    if _PROGRAM is None:
        _PROGRAM = _build_program()
    return _PROGRAM


def make_in_maps(inputs: dict) -> list[dict]:
    q = np.asarray(inputs["queries"], np.float32)
    ctxf = np.asarray(inputs["context"], np.float32)
    shared = {
        "wqT": np.asarray(inputs["wq"], np.float32).T.astype(NPBF16, order="C"),
        "wkn": np.ascontiguousarray(np.asarray(inputs["wk"], np.float32).astype(NPBF16)),
        "wvT": np.asarray(inputs["wv"], np.float32).T.astype(NPBF16, order="C"),
        "woT": np.asarray(inputs["wo"], np.float32).T.astype(NPBF16, order="C"),
        "bqr": np.ascontiguousarray(
            np.asarray(inputs["bq"], np.float32).reshape(DT, 128).T
        ),
        "bvr": np.ascontiguousarray(
            np.asarray(inputs["bv"], np.float32).reshape(DT, 128).T
        ),
        "bor": np.asarray(inputs["bo"], np.float32).reshape(1, D).copy(),
    }

    in_maps = []
    for b in range(B):
        m = dict(shared)
        cb = ctxf[b].astype(NPBF16)
        m["ctxN"] = np.ascontiguousarray(cb)
        m["ctxT"] = np.ascontiguousarray(cb.T)
        m["qT"] = q[b].T.astype(NPBF16, order="C")
        in_maps.append(m)
    return in_maps


def kernel(**inputs) -> np.ndarray:
    nc = _get_program()
    in_maps = make_in_maps(inputs)
    res = run_bass_kernel_spmd(nc, in_maps, core_ids=list(range(N_CORES)))
    return np.stack([res.results[b]["out"] for b in range(B)]).astype(np.float32)
